# revision 43
# baseline (speedup 1.0000x reference)
"""Trainium2 Bass kernel for the BoW language model head problem.

Model (per reference):
    emb = wte[x] + wpe            (B,T,C)
    h   = emb + cumsum(emb)/[1..T]
    h   = h + tanh(h@w_fc+b_fc)@w_proj + b_proj
    out = h @ w_head + b_head     (B,T,V)

Shapes: B=4, T=2048, V=50257, C=512, H=2048.

Sharding (8 cores): core i handles batch i//2 and token half i%2 —
1024 tokens x the FULL vocab.  No collectives: the only cross-half
dependency is the causal-BoW prefix base (sum of the first half's
embeddings), which each second-half core recomputes locally from its
own gather of the full embedding (under SPMD it issues those 16
indirect DMAs anyway; a mask zeroes the base on first-half cores).

The causal BoW is fused into a single pair of accumulating matmuls per
(block, C-chunk) that produce h^T directly in C-major layout:
    h^T[c,t] = sum_p E[p,c] * M[p,t] + sum_p s[p,c] * R[p,t]
with M = I + tril * recip(t) and R = recip(t) host-precomputed per
block, and s the running block-sum of E.  No PE transposes, no
post-scale.

Everything runs in bf16 operands with fp32 PSUM accumulation; logits
are written to HBM as bf16 (halves the dominant output traffic).  Head
weights stream in bf16, double-buffered in groups of 8x512 vocab
columns, the first two groups prefetched behind the embedding gathers.
"""

import ml_dtypes
import numpy as np

import concourse.bacc as bacc
import concourse.bass as bass
import concourse.mybir as mybir
import concourse.tile as tile
from concourse.bass_utils import run_bass_kernel_spmd

P = 128
B, T, V, C, H = 4, 2048, 50257, 512, 2048
NBLK = T // P          # 16 token blocks
NLOC = NBLK // 2       # 8 local token blocks per core (token-half split)
TLOC = NLOC * P        # 1024 local tokens
NCC = C // P           # 4 C chunks
NHC = H // P           # 16 H chunks
TG = 512               # token group width (MLP moving dim)
VT = 512               # vocab tile width
NVT = 99               # vocab tiles (full vocab, padded)
VT_LAST = 96           # last tile width (50257 - 98*512 = 81, pad to 96)
VPAD = (NVT - 1) * VT + VT_LAST  # 50272
GSZ = 8                # vocab tiles per weight group
N8 = 20                # trailing vocab tiles computed in fp8 DoubleRow
NBF = NVT - N8         # 79 leading tiles in bf16
# mixed head groups: 6 bf16 + 2 fp8 tiles (8 PSUM banks) while fp8
# tiles last, so the fp8 drains hide under bf16 PE time; then pure
# bf16 groups of 8 for the remainder.
GPLAN = []
_bf, _f8 = 0, NBF
while _f8 < NVT:
    GPLAN.append((list(range(_bf, _bf + 6)), [_f8, _f8 + 1]))
    _bf += 6
    _f8 += 2
while _bf < NBF:
    _n = min(GSZ, NBF - _bf)
    GPLAN.append((list(range(_bf, _bf + _n)), []))
    _bf += _n
SC_H = 256.0           # fp8 scale on h
SC_W = 1024.0          # fp8 scale on w_head tail
SC_OUT = 1.0 / (SC_H * SC_W)

F32 = mybir.dt.float32
BF16 = mybir.dt.bfloat16
F8 = mybir.dt.float8e4
I32 = mybir.dt.int32


def vt_w(v):
    return VT_LAST if v == NVT - 1 else VT


def vt_off(v):
    return v * VT


def to_bf16(x: np.ndarray) -> np.ndarray:
    """RNE-round fp32 -> bf16 (ml_dtypes.bfloat16 array)."""
    return np.ascontiguousarray(x, dtype=np.float32).astype(ml_dtypes.bfloat16)


def bf16_to_f32(u: np.ndarray) -> np.ndarray:
    return (u.astype(np.uint32) << 16).view(np.float32)


def _build_nc():
    nc = bacc.Bacc(None, target_bir_lowering=False, debug=False,
                   num_swdge_queues=4, num_devices=8)

    x_idx = nc.dram_tensor("x_idx", [P, NBLK], I32, kind="ExternalInput")
    wte = nc.dram_tensor("wte", [V, C], BF16, kind="ExternalInput")
    wpe = nc.dram_tensor("wpe", [T, C], BF16, kind="ExternalInput")
    # host pre-transposes the MLP weights into the SBUF layout so the
    # loads are one contiguous 16KB descriptor per partition
    w_fc = nc.dram_tensor("w_fc", [P, NCC * H], BF16, kind="ExternalInput")
    w_proj = nc.dram_tensor("w_proj", [P, NHC * C], BF16, kind="ExternalInput")
    w_head = nc.dram_tensor("w_head", [C, VPAD], BF16, kind="ExternalInput")
    w_head8 = nc.dram_tensor("w_head8", [C, VPAD], F8, kind="ExternalInput")
    b_fc2d = nc.dram_tensor("b_fc2d", [P, NHC], F32, kind="ExternalInput")
    b_proj2d = nc.dram_tensor("b_proj2d", [P, NCC], F32, kind="ExternalInput")
    mtri = nc.dram_tensor("mtri", [P, NLOC, P], BF16, kind="ExternalInput")
    onesr = nc.dram_tensor("onesr", [P, NLOC, P], BF16, kind="ExternalInput")
    mask = nc.dram_tensor("mask", [P, 1], F32, kind="ExternalInput")
    out = nc.dram_tensor("out", [TLOC, VPAD], BF16, kind="ExternalOutput")

    with tile.TileContext(nc) as tc:
        with tc.tile_pool(name="consts", bufs=1) as consts, \
             tc.tile_pool(name="hfp", bufs=1) as hfp, \
             tc.tile_pool(name="whp", bufs=2 * GSZ) as whp, \
             tc.tile_pool(name="htp", bufs=1) as htp:
            # idx goes over the gpsimd queue: the gathers run on the same
            # engine, so they chain with no cross-engine semaphore.
            idx_sb = consts.tile([P, NBLK], I32, tag="idx")
            nc.gpsimd.dma_start(out=idx_sb[:], in_=x_idx[:])
            mtri_sb = consts.tile([P, NLOC, P], BF16, tag="mtri")
            nc.sync.dma_start(out=mtri_sb[:], in_=mtri[:])
            onesr_sb = consts.tile([P, NLOC, P], BF16, tag="onesr")
            nc.sync.dma_start(out=onesr_sb[:], in_=onesr[:])
            mask_sb = consts.tile([P, 1], F32, tag="mask")
            nc.sync.dma_start(out=mask_sb[:], in_=mask[:])
            bfc_sb = consts.tile([P, NHC], F32, tag="bfc")
            nc.sync.dma_start(out=bfc_sb[:], in_=b_fc2d[:])
            bproj_sb = consts.tile([P, NCC], F32, tag="bproj")
            nc.sync.dma_start(out=bproj_sb[:], in_=b_proj2d[:])

            # hT holds this core's half pre-MLP (C-major); hFloc post-MLP
            # (bf16), hF8 the fp8 copy scaled by SC_H for the fp8 tail.
            hT = htp.tile([P, NCC, TLOC], BF16, tag="hT")
            hFloc = hfp.tile([P, NCC, TLOC], BF16, tag="hFloc")
            hF8 = hfp.tile([P, NCC, TLOC], F8, tag="hF8")

            wh_view = w_head.rearrange("(c p) v -> p c v", p=P)
            wh8_view = w_head8.rearrange("(c p) v -> p c v", p=P)

            def load_group(tiles):
                whs = []
                for v in tiles:
                    wh = whp.tile([P, NCC, VT], BF16, tag="wh")
                    nc.gpsimd.dma_start(
                        out=wh[:, :, :vt_w(v)],
                        in_=wh_view[:, :, vt_off(v):vt_off(v) + vt_w(v)])
                    whs.append(wh)
                return whs

            # ---- Phase B+C interleaved: embedding, causal BoW, MLP ----
            with tc.tile_pool(name="wmats", bufs=1) as wmats, \
                 tc.tile_pool(name="embp", bufs=8) as embp, \
                 tc.tile_pool(name="ebuf", bufs=1) as ebuf, \
                 tc.tile_pool(name="sp", bufs=NLOC) as sp, \
                 tc.tile_pool(name="psb", bufs=2, space="PSUM") as psb, \
                 tc.tile_pool(name="ap_", bufs=NHC) as ap_, \
                 tc.tile_pool(name="ctmp", bufs=3) as ctmp, \
                 tc.tile_pool(name="psfc", bufs=2, space="PSUM") as psfc, \
                 tc.tile_pool(name="pspj", bufs=1, space="PSUM") as pspj:
                E = ebuf.tile([P, NBLK, C], BF16, tag="E")
                # gather the OTHER half's blocks (slots 8..15) first: the
                # prefix base O must be ready before the local BoW chain.
                for j in list(range(NLOC, NBLK)) + list(range(NLOC)):
                    g = embp.tile([P, C], BF16, tag="g")
                    nc.gpsimd.indirect_dma_start(
                        out=g[:], out_offset=None, in_=wte[:],
                        in_offset=bass.IndirectOffsetOnAxis(
                            ap=idx_sb[:, j:j + 1], axis=0),
                    )
                    w = embp.tile([P, C], BF16, tag="wpe")
                    nc.sync.dma_start(out=w[:], in_=wpe[j * P:(j + 1) * P, :])
                    nc.vector.tensor_add(E[:, j, :], g[:], w[:])

                # prefetch the first two head-weight groups (bf16 parts)
                # behind the gathers on the gpsimd DMA queue
                whs_pref = [load_group(GPLAN[0][0]),
                            load_group(GPLAN[1][0])]

                # MLP weights ride the scalar engine's DMA queue: parallel
                # to the const/wpe loads (sync) and gathers (gpsimd).
                wfc_sb = wmats.tile([P, NCC, H], BF16, tag="wfc")
                nc.scalar.dma_start(
                    out=wfc_sb[:],
                    in_=w_fc.rearrange("p (c h) -> p c h", c=NCC))
                wproj_sb = wmats.tile([P, NHC, C], BF16, tag="wproj")
                nc.scalar.dma_start(
                    out=wproj_sb[:],
                    in_=w_proj.rearrange("p (hc c) -> p hc c", hc=NHC))

                # O = sum of the other half's E; s_base = O * mask
                # (mask=1 iff this core owns the second global half).
                o_cur = None
                for j in range(NLOC, NBLK):
                    o_new = sp.tile([P, C], BF16, tag="O")
                    if j == NLOC:
                        nc.vector.tensor_copy(o_new[:], E[:, j, :])
                    else:
                        nc.vector.tensor_add(o_new[:], o_cur[:], E[:, j, :])
                    o_cur = o_new
                s_base = sp.tile([P, C], BF16, tag="S")
                nc.vector.tensor_scalar_mul(s_base[:], o_cur[:], mask_sb[:, :1])

                # precompute every block's prefix tile up front: the BoW
                # matmuls then have no serial DVE<->PE ping-pong.
                s_list = [s_base]
                for j in range(NLOC - 1):
                    s_new = sp.tile([P, C], BF16, tag="S")
                    nc.vector.tensor_add(s_new[:], s_list[-1][:], E[:, j, :])
                    s_list.append(s_new)

                def bow_block(j):
                    # h^T chunks for block j: E^T@(I + tril*recip) + s^T@recip
                    ph = psb.tile([P, NCC, P], F32, tag="bow")
                    for c in range(NCC):
                        csl = slice(c * P, (c + 1) * P)
                        nc.tensor.matmul(ph[:, c, :], lhsT=E[:, j, csl],
                                         rhs=mtri_sb[:, j, :],
                                         start=True, stop=False)
                        nc.tensor.matmul(ph[:, c, :], lhsT=s_list[j][:, csl],
                                         rhs=onesr_sb[:, j, :],
                                         start=False, stop=True)
                    jsl = slice(j * P, (j + 1) * P)
                    nc.vector.tensor_copy(hT[:, 0:2, jsl], ph[:, 0:2, :])
                    nc.scalar.activation(hT[:, 2:4, jsl], ph[:, 2:4, :],
                                         mybir.ActivationFunctionType.Copy)

                def mlp_group(gidx):
                    gsl = slice(gidx * TG, (gidx + 1) * TG)
                    a_tiles = []
                    for hc in range(NHC):
                        pfc = psfc.tile([P, TG], F32, tag="fc")
                        for c in range(NCC):
                            nc.tensor.matmul(
                                pfc[:], lhsT=wfc_sb[:, c, hc * P:(hc + 1) * P],
                                rhs=hT[:, c, gsl],
                                start=(c == 0), stop=(c == NCC - 1))
                        a = ap_.tile([P, TG], BF16, tag="a")
                        nc.scalar.activation(a[:], pfc[:],
                                             mybir.ActivationFunctionType.Tanh,
                                             bias=bfc_sb[:, hc:hc + 1])
                        a_tiles.append(a)
                    pproj = pspj.tile([P, NCC, TG], F32, tag="proj")
                    for cc in range(NCC):
                        for hc in range(NHC):
                            nc.tensor.matmul(
                                pproj[:, cc, :],
                                lhsT=wproj_sb[:, hc, cc * P:(cc + 1) * P],
                                rhs=a_tiles[hc][:],
                                start=(hc == 0), stop=(hc == NHC - 1))
                        tmpc = ctmp.tile([P, TG], BF16, tag="tmpc")
                        nc.scalar.activation(tmpc[:], pproj[:, cc, :],
                                             mybir.ActivationFunctionType.Identity,
                                             bias=bproj_sb[:, cc:cc + 1])
                        nc.vector.tensor_add(hFloc[:, cc, gsl], tmpc[:],
                                             hT[:, cc, gsl])
                        nc.vector.tensor_scalar_mul(hF8[:, cc, gsl],
                                                    hFloc[:, cc, gsl], SC_H)

                for j in range(NLOC):
                    bow_block(j)
                mlp_group(0)
                mlp_group(1)

            # ---------------- Phase D: head ----------------
            with tc.tile_pool(name="stp", bufs=6) as stp, \
                 tc.tile_pool(name="whp8", bufs=N8) as whp8, \
                 tc.tile_pool(name="pso", bufs=8, space="PSUM") as pso:

                def load_group8(tiles):
                    whs = []
                    for v in tiles:
                        wh = whp8.tile([P, NCC, VT], F8, tag="wh8")
                        nc.gpsimd.dma_start(
                            out=wh[:, :, :vt_w(v)],
                            in_=wh8_view[:, :, vt_off(v):vt_off(v) + vt_w(v)])
                        whs.append(wh)
                    return whs

                def head_group(bfs, f8s, whs, whs8):
                    nb = len(bfs)
                    for j in range(NLOC):
                        jj = j * P
                        jsl = slice(j * P, (j + 1) * P)
                        psums = []
                        for _vi in range(nb + len(f8s)):
                            po = pso.tile([P, VT], F32, tag="po")
                            psums.append(po)
                        for c in range(NCC):
                            for i, v in enumerate(bfs):
                                w_ = vt_w(v)
                                nc.tensor.matmul(
                                    psums[i][:, :w_],
                                    lhsT=hFloc[:, c, jj:jj + P],
                                    rhs=whs[i][:, c, :w_],
                                    start=(c == 0), stop=(c == NCC - 1))
                        for k in range(2):
                            for i, v in enumerate(f8s):
                                w_ = vt_w(v)
                                nc.tensor.matmul(
                                    psums[nb + i][:, :w_],
                                    lhsT=hF8[:, 2 * k:2 * k + 2, jj:jj + P],
                                    rhs=whs8[i][:, 2 * k:2 * k + 2, :w_],
                                    start=(k == 0), stop=(k == 1),
                                    perf_mode=mybir.MatmulPerfMode.DoubleRow)
                        # drain spans: contiguous-column runs share a stage
                        spans = [(bfs[h0:h0 + 4], h0, False)
                                 for h0 in range(0, nb, 4)]
                        if f8s:
                            spans.append((f8s, nb, True))
                        for tiles, base, is8 in spans:
                            st = stp.tile([P, 4 * VT], BF16, tag="stage")
                            spos = 0
                            for k2, v in enumerate(tiles):
                                w_ = vt_w(v)
                                vi = base + k2
                                dst = st[:, spos:spos + w_]
                                src = psums[vi][:, :w_]
                                if vi % 2:
                                    nc.scalar.activation(
                                        dst, src,
                                        mybir.ActivationFunctionType.Copy,
                                        scale=SC_OUT if is8 else 1.0)
                                elif is8:
                                    nc.vector.tensor_scalar_mul(dst, src, SC_OUT)
                                else:
                                    nc.vector.tensor_copy(dst, src)
                                spos += w_
                            nc.sync.dma_start(
                                out=out[jsl,
                                        vt_off(tiles[0]):vt_off(tiles[0]) + spos],
                                in_=st[:, :spos])

                whs_cur = whs_pref[0]
                whs_next = whs_pref[1]
                whs8_cur = load_group8(GPLAN[0][1])
                whs8_next = load_group8(GPLAN[1][1])
                for gi, (bfs, f8s) in enumerate(GPLAN):
                    whs = whs_cur
                    whs_cur = whs_next
                    whs8 = whs8_cur
                    whs8_cur = whs8_next
                    if gi + 2 < len(GPLAN):
                        whs_next = load_group(GPLAN[gi + 2][0])
                        whs8_next = load_group8(GPLAN[gi + 2][1])
                    head_group(bfs, f8s, whs, whs8)
    nc.compile()
    return nc


_NC = None


def _get_nc():
    global _NC
    if _NC is None:
        _NC = _build_nc()
    return _NC


def make_in_maps(x, wte, wpe, w_fc, b_fc, w_proj, b_proj, w_head, b_head):
    x = np.asarray(x).astype(np.int32)
    wte_b = to_bf16(np.asarray(wte, dtype=np.float32))
    wpe_b = to_bf16(np.asarray(wpe, dtype=np.float32))
    # pre-transpose into per-partition-contiguous SBUF layout:
    # wfc_b[p, c*H + h] = w_fc[c*128 + p, h]
    wfc_b = to_bf16(np.asarray(w_fc, dtype=np.float32)
                    .reshape(NCC, P, H).transpose(1, 0, 2).reshape(P, NCC * H))
    wproj_b = to_bf16(np.asarray(w_proj, dtype=np.float32)
                      .reshape(NHC, P, C).transpose(1, 0, 2).reshape(P, NHC * C))
    whead_f = np.zeros((C, VPAD), np.float32)
    whead_f[:, :V] = np.asarray(w_head, dtype=np.float32)
    whead_b = whead_f.astype(ml_dtypes.bfloat16)
    whead_8 = (whead_f * SC_W).astype(ml_dtypes.float8_e4m3)
    b_fc = np.asarray(b_fc, dtype=np.float32)
    b_proj = np.asarray(b_proj, dtype=np.float32)

    # per-half block permutation: own half's blocks first
    orders = [list(range(th * NLOC, th * NLOC + NLOC)) +
              list(range((1 - th) * NLOC, (1 - th) * NLOC + NLOC))
              for th in range(2)]
    wpe_blocks = np.asarray(wpe_b).reshape(NBLK, P, C)
    wpe_perms = [np.ascontiguousarray(wpe_blocks[o].reshape(T, C))
                 for o in orders]
    t_idx = np.arange(1, T + 1, dtype=np.float32)
    recip_full = (1.0 / t_idx).reshape(NBLK, P)  # [NBLK, P(t)] global
    # mtri[p, j, t] = (p == t) + (p <= t) * recip ; onesr[p, j, t] = recip
    pp = np.arange(P)
    incl = (pp[:, None] <= pp[None, :]).astype(np.float32)  # [p, t]
    eye = np.eye(P, dtype=np.float32)
    mtris, onesrs = [], []
    for th in range(2):
        rec = recip_full[th * NLOC:(th + 1) * NLOC]  # [NLOC, P(t)]
        m = eye[:, None, :] + incl[:, None, :] * rec[None, :, :]
        o = np.broadcast_to(rec[None, :, :], (P, NLOC, P))
        mtris.append(to_bf16(m))
        onesrs.append(to_bf16(np.ascontiguousarray(o)))
    b_fc2d = np.ascontiguousarray(b_fc.reshape(NHC, P).T)
    b_proj2d = np.ascontiguousarray(b_proj.reshape(NCC, P).T)

    in_maps = []
    for core in range(8):
        b = core // 2
        th = core % 2
        x_idx = np.ascontiguousarray(x[b].reshape(NBLK, P)[orders[th]].T)
        in_maps.append({
            "x_idx": x_idx,
            "wte": wte_b,
            "wpe": wpe_perms[th],
            "w_fc": wfc_b,
            "w_proj": wproj_b,
            "w_head": whead_b,
            "w_head8": whead_8,
            "b_fc2d": b_fc2d,
            "b_proj2d": b_proj2d,
            "mtri": mtris[th],
            "onesr": onesrs[th],
            "mask": np.full((P, 1), float(th), np.float32),
        })
    return in_maps


def kernel(x, wte, wpe, w_fc, b_fc, w_proj, b_proj, w_head, b_head):
    b_head = np.asarray(b_head, dtype=np.float32)
    in_maps = make_in_maps(x, wte, wpe, w_fc, b_fc, w_proj, b_proj,
                           w_head, b_head)
    nc = _get_nc()
    res = run_bass_kernel_spmd(nc, in_maps, core_ids=list(range(8)))

    logits = np.empty((B, T, V), np.float32)
    for core in range(8):
        b = core // 2
        th = core % 2
        co = np.asarray(res.results[core]["out"]).view(np.uint16)
        logits[b, th * TLOC:(th + 1) * TLOC, :] = bf16_to_f32(co[:, :V])
    if b_head.any():
        logits += b_head[None, None, :]
    return logits


# revision 44
# speedup vs baseline: 1.0099x; 1.0099x over previous
"""Trainium2 Bass kernel for the BoW language model head problem.

Model (per reference):
    emb = wte[x] + wpe            (B,T,C)
    h   = emb + cumsum(emb)/[1..T]
    h   = h + tanh(h@w_fc+b_fc)@w_proj + b_proj
    out = h @ w_head + b_head     (B,T,V)

Shapes: B=4, T=2048, V=50257, C=512, H=2048.

Sharding (8 cores): core i handles batch i//2 and token half i%2 —
1024 tokens x the FULL vocab.  No collectives: the only cross-half
dependency is the causal-BoW prefix base (sum of the first half's
embeddings), which each second-half core recomputes locally from its
own gather of the full embedding (under SPMD it issues those 16
indirect DMAs anyway; a mask zeroes the base on first-half cores).

The causal BoW is fused into a single pair of accumulating matmuls per
(block, C-chunk) that produce h^T directly in C-major layout:
    h^T[c,t] = sum_p E[p,c] * M[p,t] + sum_p s[p,c] * R[p,t]
with M = I + tril * recip(t) and R = recip(t) host-precomputed per
block, and s the running block-sum of E.  No PE transposes, no
post-scale.

Everything runs in bf16 operands with fp32 PSUM accumulation; logits
are written to HBM as bf16 (halves the dominant output traffic).  Head
weights stream in bf16, double-buffered in groups of 8x512 vocab
columns, the first two groups prefetched behind the embedding gathers.
"""

import ml_dtypes
import numpy as np

import concourse.bacc as bacc
import concourse.bass as bass
import concourse.mybir as mybir
import concourse.tile as tile
from concourse.bass_utils import run_bass_kernel_spmd

P = 128
B, T, V, C, H = 4, 2048, 50257, 512, 2048
NBLK = T // P          # 16 token blocks
NLOC = NBLK // 2       # 8 local token blocks per core (token-half split)
TLOC = NLOC * P        # 1024 local tokens
NCC = C // P           # 4 C chunks
NHC = H // P           # 16 H chunks
TG = 512               # token group width (MLP moving dim)
VT = 512               # vocab tile width
NVT = 99               # vocab tiles (full vocab, padded)
VT_LAST = 96           # last tile width (50257 - 98*512 = 81, pad to 96)
VPAD = (NVT - 1) * VT + VT_LAST  # 50272
GSZ = 8                # vocab tiles per weight group
N8 = 20                # trailing vocab tiles computed in fp8 DoubleRow
NBF = NVT - N8         # 83 leading tiles in bf16
VGROUPS = [(v0, min(GSZ, NBF - v0)) for v0 in range(0, NBF, GSZ)]
VGROUPS8 = [(v0, min(GSZ, NVT - v0)) for v0 in range(NBF, NVT, GSZ)]
SC_H = 256.0           # fp8 scale on h
SC_W = 1024.0          # fp8 scale on w_head tail
SC_OUT = 1.0 / (SC_H * SC_W)

F32 = mybir.dt.float32
BF16 = mybir.dt.bfloat16
F8 = mybir.dt.float8e4
I32 = mybir.dt.int32


def vt_w(v):
    return VT_LAST if v == NVT - 1 else VT


def vt_off(v):
    return v * VT


def to_bf16(x: np.ndarray) -> np.ndarray:
    """RNE-round fp32 -> bf16 (ml_dtypes.bfloat16 array)."""
    return np.ascontiguousarray(x, dtype=np.float32).astype(ml_dtypes.bfloat16)


def bf16_to_f32(u: np.ndarray) -> np.ndarray:
    return (u.astype(np.uint32) << 16).view(np.float32)


def _build_nc():
    nc = bacc.Bacc(None, target_bir_lowering=False, debug=False,
                   num_swdge_queues=4, num_devices=8)

    x_idx = nc.dram_tensor("x_idx", [P, NBLK], I32, kind="ExternalInput")
    wte = nc.dram_tensor("wte", [V, C], BF16, kind="ExternalInput")
    wpe = nc.dram_tensor("wpe", [T, C], BF16, kind="ExternalInput")
    # host pre-transposes the MLP weights into the SBUF layout so the
    # loads are one contiguous 16KB descriptor per partition
    w_fc = nc.dram_tensor("w_fc", [P, NCC * H], BF16, kind="ExternalInput")
    w_proj = nc.dram_tensor("w_proj", [P, NHC * C], BF16, kind="ExternalInput")
    w_head = nc.dram_tensor("w_head", [C, VPAD], BF16, kind="ExternalInput")
    w_head8 = nc.dram_tensor("w_head8", [C, VPAD], F8, kind="ExternalInput")
    b_fc2d = nc.dram_tensor("b_fc2d", [P, NHC], F32, kind="ExternalInput")
    b_proj2d = nc.dram_tensor("b_proj2d", [P, NCC], F32, kind="ExternalInput")
    mtri = nc.dram_tensor("mtri", [P, NLOC, P], BF16, kind="ExternalInput")
    onesr = nc.dram_tensor("onesr", [P, NLOC, P], BF16, kind="ExternalInput")
    mask = nc.dram_tensor("mask", [P, 1], F32, kind="ExternalInput")
    out = nc.dram_tensor("out", [TLOC, VPAD], BF16, kind="ExternalOutput")

    with tile.TileContext(nc) as tc:
        with tc.tile_pool(name="consts", bufs=1) as consts, \
             tc.tile_pool(name="hfp", bufs=1) as hfp, \
             tc.tile_pool(name="whp", bufs=2 * GSZ) as whp, \
             tc.tile_pool(name="htp", bufs=1) as htp:
            # idx goes over the gpsimd queue: the gathers run on the same
            # engine, so they chain with no cross-engine semaphore.
            idx_sb = consts.tile([P, NBLK], I32, tag="idx")
            nc.gpsimd.dma_start(out=idx_sb[:], in_=x_idx[:])
            mtri_sb = consts.tile([P, NLOC, P], BF16, tag="mtri")
            nc.sync.dma_start(out=mtri_sb[:], in_=mtri[:])
            onesr_sb = consts.tile([P, NLOC, P], BF16, tag="onesr")
            nc.sync.dma_start(out=onesr_sb[:], in_=onesr[:])
            mask_sb = consts.tile([P, 1], F32, tag="mask")
            nc.sync.dma_start(out=mask_sb[:], in_=mask[:])
            bfc_sb = consts.tile([P, NHC], F32, tag="bfc")
            nc.sync.dma_start(out=bfc_sb[:], in_=b_fc2d[:])
            bproj_sb = consts.tile([P, NCC], F32, tag="bproj")
            nc.sync.dma_start(out=bproj_sb[:], in_=b_proj2d[:])

            # hT holds this core's half pre-MLP (C-major); hFloc post-MLP
            # (bf16), hF8 the fp8 copy scaled by SC_H for the fp8 tail.
            hT = htp.tile([P, NCC, TLOC], BF16, tag="hT")
            hFloc = hfp.tile([P, NCC, TLOC], BF16, tag="hFloc")
            hF8 = hfp.tile([P, NCC, TLOC], F8, tag="hF8")

            wh_view = w_head.rearrange("(c p) v -> p c v", p=P)
            wh8_view = w_head8.rearrange("(c p) v -> p c v", p=P)

            def load_group(v0, nv, eng=None):
                eng = eng or nc.gpsimd
                whs = []
                for v in range(v0, v0 + nv):
                    wh = whp.tile([P, NCC, VT], BF16, tag="wh")
                    eng.dma_start(
                        out=wh[:, :, :vt_w(v)],
                        in_=wh_view[:, :, vt_off(v):vt_off(v) + vt_w(v)])
                    whs.append(wh)
                return whs

            # ---- Phase B+C interleaved: embedding, causal BoW, MLP ----
            with tc.tile_pool(name="wmats", bufs=1) as wmats, \
                 tc.tile_pool(name="embp", bufs=8) as embp, \
                 tc.tile_pool(name="ebuf", bufs=1) as ebuf, \
                 tc.tile_pool(name="sp", bufs=NLOC) as sp, \
                 tc.tile_pool(name="psb", bufs=2, space="PSUM") as psb, \
                 tc.tile_pool(name="ap_", bufs=NHC) as ap_, \
                 tc.tile_pool(name="ctmp", bufs=3) as ctmp, \
                 tc.tile_pool(name="psfc", bufs=2, space="PSUM") as psfc, \
                 tc.tile_pool(name="pspj", bufs=1, space="PSUM") as pspj:
                E = ebuf.tile([P, NBLK, C], BF16, tag="E")
                # gather the OTHER half's blocks (slots 8..15) first: the
                # prefix base O must be ready before the local BoW chain.
                for j in list(range(NLOC, NBLK)) + list(range(NLOC)):
                    g = embp.tile([P, C], BF16, tag="g")
                    nc.gpsimd.indirect_dma_start(
                        out=g[:], out_offset=None, in_=wte[:],
                        in_offset=bass.IndirectOffsetOnAxis(
                            ap=idx_sb[:, j:j + 1], axis=0),
                    )
                    w = embp.tile([P, C], BF16, tag="wpe")
                    nc.sync.dma_start(out=w[:], in_=wpe[j * P:(j + 1) * P, :])
                    nc.vector.tensor_add(E[:, j, :], g[:], w[:])

                # prefetch the first two head-weight groups behind the
                # gathers on the gpsimd DMA queue
                whs_pref = [load_group(*VGROUPS[0]),
                            load_group(*VGROUPS[1])]

                # MLP weights ride the scalar engine's DMA queue: parallel
                # to the const/wpe loads (sync) and gathers (gpsimd).
                wfc_sb = wmats.tile([P, NCC, H], BF16, tag="wfc")
                nc.scalar.dma_start(
                    out=wfc_sb[:],
                    in_=w_fc.rearrange("p (c h) -> p c h", c=NCC))
                wproj_sb = wmats.tile([P, NHC, C], BF16, tag="wproj")
                nc.scalar.dma_start(
                    out=wproj_sb[:],
                    in_=w_proj.rearrange("p (hc c) -> p hc c", hc=NHC))

                # O = sum of the other half's E; s_base = O * mask
                # (mask=1 iff this core owns the second global half).
                o_cur = None
                for j in range(NLOC, NBLK):
                    o_new = sp.tile([P, C], BF16, tag="O")
                    if j == NLOC:
                        nc.vector.tensor_copy(o_new[:], E[:, j, :])
                    else:
                        nc.vector.tensor_add(o_new[:], o_cur[:], E[:, j, :])
                    o_cur = o_new
                s_base = sp.tile([P, C], BF16, tag="S")
                nc.vector.tensor_scalar_mul(s_base[:], o_cur[:], mask_sb[:, :1])

                # precompute every block's prefix tile up front: the BoW
                # matmuls then have no serial DVE<->PE ping-pong.
                s_list = [s_base]
                for j in range(NLOC - 1):
                    s_new = sp.tile([P, C], BF16, tag="S")
                    nc.vector.tensor_add(s_new[:], s_list[-1][:], E[:, j, :])
                    s_list.append(s_new)

                def bow_block(j):
                    # h^T chunks for block j: E^T@(I + tril*recip) + s^T@recip
                    ph = psb.tile([P, NCC, P], F32, tag="bow")
                    for c in range(NCC):
                        csl = slice(c * P, (c + 1) * P)
                        nc.tensor.matmul(ph[:, c, :], lhsT=E[:, j, csl],
                                         rhs=mtri_sb[:, j, :],
                                         start=True, stop=False)
                        nc.tensor.matmul(ph[:, c, :], lhsT=s_list[j][:, csl],
                                         rhs=onesr_sb[:, j, :],
                                         start=False, stop=True)
                    jsl = slice(j * P, (j + 1) * P)
                    nc.vector.tensor_copy(hT[:, 0:2, jsl], ph[:, 0:2, :])
                    nc.scalar.activation(hT[:, 2:4, jsl], ph[:, 2:4, :],
                                         mybir.ActivationFunctionType.Copy)

                def mlp_group(gidx):
                    gsl = slice(gidx * TG, (gidx + 1) * TG)
                    a_tiles = []
                    for hc in range(NHC):
                        pfc = psfc.tile([P, TG], F32, tag="fc")
                        for c in range(NCC):
                            nc.tensor.matmul(
                                pfc[:], lhsT=wfc_sb[:, c, hc * P:(hc + 1) * P],
                                rhs=hT[:, c, gsl],
                                start=(c == 0), stop=(c == NCC - 1))
                        a = ap_.tile([P, TG], BF16, tag="a")
                        nc.scalar.activation(a[:], pfc[:],
                                             mybir.ActivationFunctionType.Tanh,
                                             bias=bfc_sb[:, hc:hc + 1])
                        a_tiles.append(a)
                    pproj = pspj.tile([P, NCC, TG], F32, tag="proj")
                    for cc in range(NCC):
                        for hc in range(NHC):
                            nc.tensor.matmul(
                                pproj[:, cc, :],
                                lhsT=wproj_sb[:, hc, cc * P:(cc + 1) * P],
                                rhs=a_tiles[hc][:],
                                start=(hc == 0), stop=(hc == NHC - 1))
                        tmpc = ctmp.tile([P, TG], BF16, tag="tmpc")
                        nc.scalar.activation(tmpc[:], pproj[:, cc, :],
                                             mybir.ActivationFunctionType.Identity,
                                             bias=bproj_sb[:, cc:cc + 1])
                        nc.vector.tensor_add(hFloc[:, cc, gsl], tmpc[:],
                                             hT[:, cc, gsl])
                        nc.vector.tensor_scalar_mul(hF8[:, cc, gsl],
                                                    hFloc[:, cc, gsl], SC_H)

                for j in range(NLOC):
                    bow_block(j)
                mlp_group(0)
                mlp_group(1)

            # ---------------- Phase D: head ----------------
            with tc.tile_pool(name="stp", bufs=4) as stp, \
                 tc.tile_pool(name="whp8", bufs=N8) as whp8, \
                 tc.tile_pool(name="pso", bufs=8, space="PSUM") as pso:

                def load_group8(v0, nv):
                    whs = []
                    for v in range(v0, v0 + nv):
                        wh = whp8.tile([P, NCC, VT], F8, tag="wh8")
                        nc.gpsimd.dma_start(
                            out=wh[:, :, :vt_w(v)],
                            in_=wh8_view[:, :, vt_off(v):vt_off(v) + vt_w(v)])
                        whs.append(wh)
                    return whs

                def head_group(v0, nv, whs, fp8):
                    halves = [(h0, min(4, nv - h0)) for h0 in range(0, nv, 4)]
                    for j in range(NLOC):
                        jj = j * P
                        jsl = slice(j * P, (j + 1) * P)
                        stages = []
                        for h0, hn in halves:
                            st = stp.tile([P, 4 * VT], BF16, tag="stage")
                            stages.append(st)
                        psums = []
                        for _vi in range(nv):
                            po = pso.tile([P, VT], F32, tag="po")
                            psums.append(po)
                        if fp8:
                            for k in range(2):
                                for vi in range(nv):
                                    w_ = vt_w(v0 + vi)
                                    nc.tensor.matmul(
                                        psums[vi][:, :w_],
                                        lhsT=hF8[:, 2 * k:2 * k + 2, jj:jj + P],
                                        rhs=whs[vi][:, 2 * k:2 * k + 2, :w_],
                                        start=(k == 0), stop=(k == 1),
                                        perf_mode=mybir.MatmulPerfMode.DoubleRow)
                        else:
                            for c in range(NCC):
                                for vi in range(nv):
                                    w_ = vt_w(v0 + vi)
                                    nc.tensor.matmul(
                                        psums[vi][:, :w_],
                                        lhsT=hFloc[:, c, jj:jj + P],
                                        rhs=whs[vi][:, c, :w_],
                                        start=(c == 0), stop=(c == NCC - 1))
                        for hi, (h0, hn) in enumerate(halves):
                            spos = 0
                            for vi in range(h0, h0 + hn):
                                w_ = vt_w(v0 + vi)
                                dst = stages[hi][:, spos:spos + w_]
                                if vi % 2:
                                    nc.scalar.activation(
                                        dst, psums[vi][:, :w_],
                                        mybir.ActivationFunctionType.Copy,
                                        scale=SC_OUT if fp8 else 1.0)
                                else:
                                    if fp8:
                                        nc.vector.tensor_scalar_mul(
                                            dst, psums[vi][:, :w_], SC_OUT)
                                    else:
                                        nc.vector.tensor_copy(
                                            dst, psums[vi][:, :w_])
                                spos += w_
                            nc.sync.dma_start(
                                out=out[jsl,
                                        vt_off(v0 + h0):vt_off(v0 + h0) + spos],
                                in_=stages[hi][:, :spos])

                whs_cur = whs_pref[0]
                whs_next = whs_pref[1]
                whs8 = None
                for gi, (v0, nv) in enumerate(VGROUPS):
                    whs = whs_cur
                    whs_cur = whs_next
                    if gi + 2 < len(VGROUPS):
                        whs_next = load_group(*VGROUPS[gi + 2])
                    elif whs8 is None:
                        whs8 = [load_group8(*g8) for g8 in VGROUPS8]
                    head_group(v0, nv, whs, fp8=False)
                for gi8, (v0, nv) in enumerate(VGROUPS8):
                    head_group(v0, nv, whs8[gi8], fp8=True)
    nc.compile()
    return nc


_NC = None


def _get_nc():
    global _NC
    if _NC is None:
        _NC = _build_nc()
    return _NC


def make_in_maps(x, wte, wpe, w_fc, b_fc, w_proj, b_proj, w_head, b_head):
    x = np.asarray(x).astype(np.int32)
    wte_b = to_bf16(np.asarray(wte, dtype=np.float32))
    wpe_b = to_bf16(np.asarray(wpe, dtype=np.float32))
    # pre-transpose into per-partition-contiguous SBUF layout:
    # wfc_b[p, c*H + h] = w_fc[c*128 + p, h]
    wfc_b = to_bf16(np.asarray(w_fc, dtype=np.float32)
                    .reshape(NCC, P, H).transpose(1, 0, 2).reshape(P, NCC * H))
    wproj_b = to_bf16(np.asarray(w_proj, dtype=np.float32)
                      .reshape(NHC, P, C).transpose(1, 0, 2).reshape(P, NHC * C))
    whead_f = np.zeros((C, VPAD), np.float32)
    whead_f[:, :V] = np.asarray(w_head, dtype=np.float32)
    whead_b = whead_f.astype(ml_dtypes.bfloat16)
    whead_8 = (whead_f * SC_W).astype(ml_dtypes.float8_e4m3)
    b_fc = np.asarray(b_fc, dtype=np.float32)
    b_proj = np.asarray(b_proj, dtype=np.float32)

    # per-half block permutation: own half's blocks first
    orders = [list(range(th * NLOC, th * NLOC + NLOC)) +
              list(range((1 - th) * NLOC, (1 - th) * NLOC + NLOC))
              for th in range(2)]
    wpe_blocks = np.asarray(wpe_b).reshape(NBLK, P, C)
    wpe_perms = [np.ascontiguousarray(wpe_blocks[o].reshape(T, C))
                 for o in orders]
    t_idx = np.arange(1, T + 1, dtype=np.float32)
    recip_full = (1.0 / t_idx).reshape(NBLK, P)  # [NBLK, P(t)] global
    # mtri[p, j, t] = (p == t) + (p <= t) * recip ; onesr[p, j, t] = recip
    pp = np.arange(P)
    incl = (pp[:, None] <= pp[None, :]).astype(np.float32)  # [p, t]
    eye = np.eye(P, dtype=np.float32)
    mtris, onesrs = [], []
    for th in range(2):
        rec = recip_full[th * NLOC:(th + 1) * NLOC]  # [NLOC, P(t)]
        m = eye[:, None, :] + incl[:, None, :] * rec[None, :, :]
        o = np.broadcast_to(rec[None, :, :], (P, NLOC, P))
        mtris.append(to_bf16(m))
        onesrs.append(to_bf16(np.ascontiguousarray(o)))
    b_fc2d = np.ascontiguousarray(b_fc.reshape(NHC, P).T)
    b_proj2d = np.ascontiguousarray(b_proj.reshape(NCC, P).T)

    in_maps = []
    for core in range(8):
        b = core // 2
        th = core % 2
        x_idx = np.ascontiguousarray(x[b].reshape(NBLK, P)[orders[th]].T)
        in_maps.append({
            "x_idx": x_idx,
            "wte": wte_b,
            "wpe": wpe_perms[th],
            "w_fc": wfc_b,
            "w_proj": wproj_b,
            "w_head": whead_b,
            "w_head8": whead_8,
            "b_fc2d": b_fc2d,
            "b_proj2d": b_proj2d,
            "mtri": mtris[th],
            "onesr": onesrs[th],
            "mask": np.full((P, 1), float(th), np.float32),
        })
    return in_maps


def kernel(x, wte, wpe, w_fc, b_fc, w_proj, b_proj, w_head, b_head):
    b_head = np.asarray(b_head, dtype=np.float32)
    in_maps = make_in_maps(x, wte, wpe, w_fc, b_fc, w_proj, b_proj,
                           w_head, b_head)
    nc = _get_nc()
    res = run_bass_kernel_spmd(nc, in_maps, core_ids=list(range(8)))

    logits = np.empty((B, T, V), np.float32)
    for core in range(8):
        b = core // 2
        th = core % 2
        co = np.asarray(res.results[core]["out"]).view(np.uint16)
        logits[b, th * TLOC:(th + 1) * TLOC, :] = bf16_to_f32(co[:, :V])
    if b_head.any():
        logits += b_head[None, None, :]
    return logits
